# revision 1
# baseline (speedup 1.0000x reference)
"""Bayesian dense layer (per-sample reparameterized weights) on 8 TRN2 NeuronCores.

Computes out[b] = x[b] @ (W[b] * softplus(log_std) + mean) + bias for
B=512, IN=OUT=1024, data-parallel over the batch axis (64 rows per core).

Device algorithm per core (batch slice of BPC=64 rows):
  - layout: partition dim = i (contraction index), free dim = o; i-blocks of
    256 rows with i = blk*256 + 2p + jj so each per-(partition,row) W read is
    8 KB contiguous and each per-row-block DMA is a fully sequential 1 MiB
  - S = softplus(log_std) is precomputed on host (bf16); x arrives
    pre-transposed as xT [IN, BPC] in fp32 (mean term) + bf16 (sample term)
  - mean term: psum_mean[64, OUT] = xT.T @ mean + ones.T @ bias, once at
    full PE width in fp32 (~15 us), copied to SBUF
  - per-sample term: W tiles [128, 2, OUT] stream 16-deep (deep pipelining
    lifts per-core HBM read rate from ~335 to ~408 GB/s); the S multiply
    runs on DVE (3/4 of tiles) and GpSimd (1/4) writing bf16; per row b a
    [1, OUT] PSUM accumulator (matmul outputs must start at partition
    0/32/64) collects 8 bf16 matmuls (bf16 streams the PE at 1 col/cycle vs
    1/4 for fp32); finished rows scatter back to partition b of an SBUF
    collector via small SBUF->SBUF DMAs
  - merge: one DVE add of the two [64, OUT] terms, one DMA to DRAM
The kernel is HBM-bound: it streams 256 MiB of W per core.
"""

import os
import sys

for _p in ("/root/.axon_site", "/root/.axon_site/_ro/trn_rl_repo",
           "/root/.axon_site/_ro/pypackages"):
    if os.path.isdir(_p) and _p not in sys.path:
        sys.path.append(_p)

import numpy as np

import concourse.bass as bass
import concourse.mybir as mybir
import concourse.tile as tile
from concourse import bacc
from concourse.bass_utils import run_bass_kernel_spmd

B, IN, OUT = 512, 1024, 1024
NCORES = 8
BPC = B // NCORES  # batch rows per core

_BUILT = {}


def build_bass(bpc=BPC, in_dim=IN, out_dim=OUT, wbufs=16):
    """Build the per-core Bass module (all cores run the same program)."""
    key = (bpc, in_dim, out_dim, wbufs)
    if key in _BUILT:
        return _BUILT[key]

    f32 = mybir.dt.float32
    bf16 = mybir.dt.bfloat16
    nib = in_dim // 256           # i-blocks of 256 (2 i-rows per partition)
    nch = max(1, out_dim // 512)  # output chunks per matmul (N<=512)
    chunk = out_dim // nch

    nc = bacc.Bacc("TRN2", target_bir_lowering=False, debug=False,
                   num_devices=NCORES)

    xT = nc.dram_tensor("xT", [in_dim, bpc], f32, kind="ExternalInput").ap()
    xTh = nc.dram_tensor("xTh", [in_dim, bpc], bf16, kind="ExternalInput").ap()
    W = nc.dram_tensor("W", [bpc, in_dim, out_dim], f32,
                       kind="ExternalInput").ap()
    S = nc.dram_tensor("S", [in_dim, out_dim], bf16, kind="ExternalInput").ap()
    mean = nc.dram_tensor("mean", [in_dim, out_dim], f32,
                          kind="ExternalInput").ap()
    bias = nc.dram_tensor("bias", [1, out_dim], f32, kind="ExternalInput").ap()
    out = nc.dram_tensor("out", [bpc, out_dim], f32,
                         kind="ExternalOutput").ap()

    with tile.TileContext(nc) as tc:
        with (
            tc.tile_pool(name="singles", bufs=1) as singles,
            tc.tile_pool(name="wpool", bufs=wbufs) as wpool,
            tc.tile_pool(name="hpool", bufs=6) as hpool,
            tc.tile_pool(name="opool", bufs=2) as opool,
            tc.tile_pool(name="psum", bufs=1, space="PSUM") as psum,
            tc.tile_pool(name="psrow", bufs=3, space="PSUM") as psrow,
        ):
            xT_sb = singles.tile([128, nib, 2, bpc], f32)
            nc.sync.dma_start(
                out=xT_sb,
                in_=xT.rearrange("(ib p jj) b -> p ib jj b", p=128, jj=2))
            xTh_sb = singles.tile([128, nib, 2, bpc], bf16)
            nc.sync.dma_start(
                out=xTh_sb,
                in_=xTh.rearrange("(ib p jj) b -> p ib jj b", p=128, jj=2))
            S_sb = singles.tile([128, nib, 2, out_dim], bf16)
            nc.sync.dma_start(
                out=S_sb,
                in_=S.rearrange("(ib p jj) o -> p ib jj o", p=128, jj=2))
            bias_sb = singles.tile([1, out_dim], f32)
            nc.sync.dma_start(out=bias_sb, in_=bias)
            ones = singles.tile([1, bpc], f32)
            nc.vector.memset(ones, 1.0)

            # ── mean term at full PE width: mb_sb = xT.T @ mean + bias ──
            # mean tiles share wpool slots with W tiles (same shape/tag)
            acc_m = psum.tile([bpc, out_dim], f32)
            for ib in range(nib):
                m_t = wpool.tile([128, 2, out_dim], f32, tag="w",
                                 name=f"m_t{ib}")
                nc.sync.dma_start(
                    out=m_t,
                    in_=mean[ib * 256:(ib + 1) * 256, :]
                    .rearrange("(p jj) o -> p jj o", jj=2))
                for jj in range(2):
                    for n in range(nch):
                        nc.tensor.matmul(
                            acc_m[:, n * chunk:(n + 1) * chunk],
                            xT_sb[:, ib, jj, :],
                            m_t[:, jj, n * chunk:(n + 1) * chunk],
                            start=(ib == 0 and jj == 0), stop=False,
                            skip_group_check=True)
            for n in range(nch):
                nc.tensor.matmul(
                    acc_m[:, n * chunk:(n + 1) * chunk],
                    ones,
                    bias_sb[:, n * chunk:(n + 1) * chunk],
                    start=False, stop=True, skip_group_check=True)
            mb_sb = singles.tile([bpc, out_dim], f32)
            nc.scalar.copy(mb_sb, acc_m)

            # ── per-sample term, collected per row into wt_sb ──
            wt_sb = singles.tile([bpc, out_dim], f32)
            for b in range(bpc):
                acc = psrow.tile([1, out_dim], f32, tag="acc", name=f"acc{b}")
                for ib in range(nib):
                    w_t = wpool.tile([128, 2, out_dim], f32, tag="w",
                                     name=f"w_t{b}_{ib}")
                    nc.sync.dma_start(
                        out=w_t,
                        in_=W[b, ib * 256:(ib + 1) * 256, :]
                        .rearrange("(p jj) o -> p jj o", jj=2))
                    w_h = hpool.tile([128, 2, out_dim], bf16, tag="wh",
                                     name=f"w_h{b}_{ib}")
                    # spread the S multiply: every 4th tile on GpSimd
                    mul_eng = (nc.gpsimd if (b * nib + ib) % 4 == 3
                               else nc.vector)
                    for jj in range(2):
                        mul_eng.tensor_mul(w_h[:, jj, :], w_t[:, jj, :],
                                           S_sb[:, ib, jj, :])
                        for n in range(nch):
                            nc.tensor.matmul(
                                acc[:, n * chunk:(n + 1) * chunk],
                                xTh_sb[:, ib, jj, b:b + 1],
                                w_h[:, jj, n * chunk:(n + 1) * chunk],
                                start=(ib == 0 and jj == 0),
                                stop=(ib == nib - 1 and jj == 1),
                                skip_group_check=True)
                row = opool.tile([1, out_dim], f32, tag="row",
                                 name=f"row{b}")
                nc.scalar.copy(row, acc)
                nc.scalar.dma_start(out=wt_sb[b:b + 1, :], in_=row)

            # ── merge and write out ──
            nc.vector.tensor_add(wt_sb, wt_sb, mb_sb)
            nc.sync.dma_start(out=out, in_=wt_sb)

    nc.finalize()
    _BUILT[key] = nc
    return nc


def _softplus(x):
    return np.logaddexp(0.0, x.astype(np.float32)).astype(np.float32)


def _run(x, W, mean, log_std, bias, **kwargs):
    import ml_dtypes
    x = np.ascontiguousarray(x, dtype=np.float32)
    W = np.ascontiguousarray(W, dtype=np.float32)
    mean = np.ascontiguousarray(mean, dtype=np.float32)
    bias2 = np.ascontiguousarray(bias, dtype=np.float32).reshape(1, OUT)
    S = _softplus(log_std).astype(ml_dtypes.bfloat16)

    nc = build_bass()
    in_maps = []
    for c in range(NCORES):
        sl = slice(c * BPC, (c + 1) * BPC)
        xTc = np.ascontiguousarray(x[sl].T)
        in_maps.append({
            "xT": xTc,
            "xTh": xTc.astype(ml_dtypes.bfloat16),
            "W": W[sl],
            "S": S,
            "mean": mean,
            "bias": bias2,
        })
    res = run_bass_kernel_spmd(nc, in_maps, core_ids=list(range(NCORES)),
                               **kwargs)
    out = np.concatenate([res.results[c]["out"] for c in range(NCORES)],
                         axis=0)
    return out, res


def kernel(x, W, mean, log_std, bias):
    return _run(x, W, mean, log_std, bias)[0]



# revision 2
# speedup vs baseline: 1.8494x; 1.8494x over previous
"""Bayesian dense layer (per-sample reparameterized weights) on 8 TRN2 NeuronCores.

Computes out[b] = x[b] @ (W[b] * softplus(log_std) + mean) + bias for
B=512, IN=OUT=1024, data-parallel over the batch axis (64 rows per core).

v2: W/mean/S/x all uploaded as bf16 (rel-err budget 2e-2 easily absorbs the
~0.1% quantization; measured total err stays ~3e-3).  This halves the HBM
traffic vs the f32 baseline: the kernel streams 128 MiB of W per core and is
HBM-bound at ~358 GB/s/core -> ~390 us floor.

Device algorithm per core (batch slice of BPC=64 rows):
  - layout: partition dim = i (contraction index), free dim = o; i-blocks of
    256 rows with i = blk*256 + 2p + jj so each per-(partition,row) W read is
    4 KB contiguous and each per-row-block DMA is a fully sequential 512 KiB
  - S = softplus(log_std) is precomputed on host (bf16); x arrives
    pre-transposed as xTh [IN, BPC] bf16
  - mean term: psum_mean[64, OUT] = xTh.T @ mean + ones.T @ bias, once at
    full PE width (~4 us), copied to SBUF
  - per-sample term: W tiles [128, 2, OUT] bf16 stream 16-deep; the S
    multiply runs on DVE in 2x packed mode (bf16 in/out, ~1.1 us/tile, fully
    hidden under the DMA); per row b a [1, OUT] PSUM accumulator collects 8
    bf16 matmuls; finished rows scatter back to partition b of an SBUF
    collector via small SBUF->SBUF DMAs
  - merge: one DVE add of the two [64, OUT] terms, one DMA to DRAM
"""

import os
import sys

for _p in ("/root/.axon_site", "/root/.axon_site/_ro/trn_rl_repo",
           "/root/.axon_site/_ro/pypackages"):
    if os.path.isdir(_p) and _p not in sys.path:
        sys.path.append(_p)

import numpy as np

import concourse.bass as bass
import concourse.mybir as mybir
import concourse.tile as tile
from concourse import bacc
from concourse.bass_utils import run_bass_kernel_spmd

B, IN, OUT = 512, 1024, 1024
NCORES = 8
BPC = B // NCORES  # batch rows per core

_BUILT = {}


def build_bass(bpc=BPC, in_dim=IN, out_dim=OUT, wbufs=16, hbufs=6):
    """Build the per-core Bass module (all cores run the same program)."""
    key = (bpc, in_dim, out_dim, wbufs, hbufs)
    if key in _BUILT:
        return _BUILT[key]

    f32 = mybir.dt.float32
    bf16 = mybir.dt.bfloat16
    nib = in_dim // 256           # i-blocks of 256 (2 i-rows per partition)
    nch = max(1, out_dim // 512)  # output chunks per matmul (N<=512)
    chunk = out_dim // nch

    nc = bacc.Bacc("TRN2", target_bir_lowering=False, debug=False,
                   num_devices=NCORES)

    xTh = nc.dram_tensor("xTh", [in_dim, bpc], bf16, kind="ExternalInput").ap()
    W = nc.dram_tensor("W", [bpc, in_dim, out_dim], bf16,
                       kind="ExternalInput").ap()
    S = nc.dram_tensor("S", [in_dim, out_dim], bf16, kind="ExternalInput").ap()
    mean = nc.dram_tensor("mean", [in_dim, out_dim], bf16,
                          kind="ExternalInput").ap()
    bias = nc.dram_tensor("bias", [1, out_dim], f32, kind="ExternalInput").ap()
    out = nc.dram_tensor("out", [bpc, out_dim], f32,
                         kind="ExternalOutput").ap()

    with tile.TileContext(nc) as tc:
        with (
            tc.tile_pool(name="singles", bufs=1) as singles,
            tc.tile_pool(name="wpool", bufs=wbufs) as wpool,
            tc.tile_pool(name="hpool", bufs=hbufs) as hpool,
            tc.tile_pool(name="opool", bufs=2) as opool,
            tc.tile_pool(name="psum", bufs=1, space="PSUM") as psum,
            tc.tile_pool(name="psrow", bufs=3, space="PSUM") as psrow,
        ):
            xTh_sb = singles.tile([128, nib, 2, bpc], bf16)
            nc.sync.dma_start(
                out=xTh_sb,
                in_=xTh.rearrange("(ib p jj) b -> p ib jj b", p=128, jj=2))
            S_sb = singles.tile([128, nib, 2, out_dim], bf16)
            nc.sync.dma_start(
                out=S_sb,
                in_=S.rearrange("(ib p jj) o -> p ib jj o", p=128, jj=2))
            bias_sb = singles.tile([1, out_dim], f32)
            nc.sync.dma_start(out=bias_sb, in_=bias)
            ones = singles.tile([1, bpc], f32)
            nc.vector.memset(ones, 1.0)

            # ── mean term at full PE width: mb_sb = xTh.T @ mean + bias ──
            # mean tiles share wpool slots with W tiles (same shape/tag)
            acc_m = psum.tile([bpc, out_dim], f32)
            for ib in range(nib):
                m_t = wpool.tile([128, 2, out_dim], bf16, tag="w",
                                 name=f"m_t{ib}")
                nc.sync.dma_start(
                    out=m_t,
                    in_=mean[ib * 256:(ib + 1) * 256, :]
                    .rearrange("(p jj) o -> p jj o", jj=2))
                for jj in range(2):
                    for n in range(nch):
                        nc.tensor.matmul(
                            acc_m[:, n * chunk:(n + 1) * chunk],
                            xTh_sb[:, ib, jj, :],
                            m_t[:, jj, n * chunk:(n + 1) * chunk],
                            start=(ib == 0 and jj == 0), stop=False,
                            skip_group_check=True)
            for n in range(nch):
                nc.tensor.matmul(
                    acc_m[:, n * chunk:(n + 1) * chunk],
                    ones,
                    bias_sb[:, n * chunk:(n + 1) * chunk],
                    start=False, stop=True, skip_group_check=True)
            mb_sb = singles.tile([bpc, out_dim], f32)
            nc.scalar.copy(mb_sb, acc_m)

            # ── per-sample term, collected per row into wt_sb ──
            wt_sb = singles.tile([bpc, out_dim], f32)
            for b in range(bpc):
                acc = psrow.tile([1, out_dim], f32, tag="acc", name=f"acc{b}")
                for ib in range(nib):
                    w_t = wpool.tile([128, 2, out_dim], bf16, tag="w",
                                     name=f"w_t{b}_{ib}")
                    nc.sync.dma_start(
                        out=w_t,
                        in_=W[b, ib * 256:(ib + 1) * 256, :]
                        .rearrange("(p jj) o -> p jj o", jj=2))
                    w_h = hpool.tile([128, 2, out_dim], bf16, tag="wh",
                                     name=f"w_h{b}_{ib}")
                    # bf16 x bf16 -> bf16 runs in DVE 2x packed mode; one op
                    # over the whole [128, 2*OUT] tile
                    nc.vector.tensor_mul(w_h, w_t, S_sb[:, ib])
                    for jj in range(2):
                        for n in range(nch):
                            nc.tensor.matmul(
                                acc[:, n * chunk:(n + 1) * chunk],
                                xTh_sb[:, ib, jj, b:b + 1],
                                w_h[:, jj, n * chunk:(n + 1) * chunk],
                                start=(ib == 0 and jj == 0),
                                stop=(ib == nib - 1 and jj == 1),
                                skip_group_check=True)
                row = opool.tile([1, out_dim], f32, tag="row",
                                 name=f"row{b}")
                nc.scalar.copy(row, acc)
                nc.scalar.dma_start(out=wt_sb[b:b + 1, :], in_=row)

            # ── merge and write out ──
            nc.vector.tensor_add(wt_sb, wt_sb, mb_sb)
            nc.sync.dma_start(out=out, in_=wt_sb)

    nc.finalize()
    _BUILT[key] = nc
    return nc


def _softplus(x):
    return np.logaddexp(0.0, x.astype(np.float32)).astype(np.float32)


def _run(x, W, mean, log_std, bias, **kwargs):
    import ml_dtypes
    bf16 = ml_dtypes.bfloat16
    x = np.ascontiguousarray(x, dtype=np.float32)
    Wh = np.ascontiguousarray(W, dtype=np.float32).astype(bf16)
    mean_h = np.ascontiguousarray(mean, dtype=np.float32).astype(bf16)
    bias2 = np.ascontiguousarray(bias, dtype=np.float32).reshape(1, OUT)
    S = _softplus(log_std).astype(bf16)

    nc = build_bass()
    in_maps = []
    for c in range(NCORES):
        sl = slice(c * BPC, (c + 1) * BPC)
        in_maps.append({
            "xTh": np.ascontiguousarray(x[sl].T).astype(bf16),
            "W": Wh[sl],
            "S": S,
            "mean": mean_h,
            "bias": bias2,
        })
    res = run_bass_kernel_spmd(nc, in_maps, core_ids=list(range(NCORES)),
                               **kwargs)
    out = np.concatenate([res.results[c]["out"] for c in range(NCORES)],
                         axis=0)
    return out, res


def kernel(x, W, mean, log_std, bias):
    return _run(x, W, mean, log_std, bias)[0]
